# revision 17
# baseline (speedup 1.0000x reference)
"""PolaLinearAttention TRN2 kernel.

Math (reference):
    x  = LayerNorm(f)*g + b_ln                  [B,T,H], T=3, H=768
    Q  = x@Wq.T+bq ; K = x@Wk.T+bk ; V = x@Wv.T+bv
    Sp = softmax(relu(Q)relu(K).T * H^-.5) ; Sn = softmax(relu(-Q)relu(-K).T * H^-.5)
    out = x + (Sp - Sn)@V@Wo.T + bo

Device strategy (pure batch data-parallel over 8 cores):
  Host folds:
    - g (ln scale) and H^-0.25 into projection weights
    - LN mean-centering into column-centered weights (rank-1 update)
    - Wo into Wv (V2 = V@Wo.T directly); V-path biases cancel through
      softmax row-sums (sum_j Sp = sum_j Sn = 1)
    - per-token LN rstd r applied via relu positive-homogeneity:
      S[i,j] *= r_i*r_j after the QK dot products; r_j folded into the
      attention coefficients of the V2 combination
    - a bf16 copy of features is shipped so the feature transpose runs
      on the DMA xbar (16-bit only) straight out of DRAM
  Device per 128-batch group:
    DMA F (fp32) + 6 xbar-transposed bf16 chunks -> PE matmul
    [Q1|K1] and [V21|mu] = F^T.T @ Wcat -> relus (DVE, from PSUM) ->
    9x2 elementwise products (gpsimd) + row-sums (ACT accum) ->
    softmax small ops (DVE+ACT) -> scalar_tensor_tensor V2 combination
    seeded with xn (residual) -> DMA out.
"""

import sys

sys.path.insert(0, "/opt/trn_rl_repo")

import numpy as np
import ml_dtypes
from contextlib import ExitStack

import concourse.bass as bass
import concourse.bacc as bacc
import concourse.tile as tile
from concourse import mybir
from concourse.bass_utils import run_bass_kernel_spmd

DT = mybir.dt
ALU = mybir.AluOpType
ACT = mybir.ActivationFunctionType

H = 768
T = 3
B = 65536
NCORES = 8
BC = B // NCORES  # 8192 batches per core
G = 128           # batches per group
EPS = 1e-5
NCH = H // 128    # 6 k-chunks
WQK = 2 * H                # 1536 cols -> psum tile A (3 banks exactly)
WVM = H + 2                # 770 cols: V2 | mu | pad -> psum tile B
WALL = WQK + WVM           # 2306 weight columns total
QK_SLICES = [(0, 512), (512, 1024), (1024, 1536)]
VM_SLICES = [(0, 512), (512, 770)]


def build_nc(groups: int) -> bass.Bass:
    nc = bacc.Bacc(None)
    f_in = nc.dram_tensor("features", [BC, T, H], DT.float32, kind="ExternalInput")
    fb_in = nc.dram_tensor("fbf", [BC, T, H], DT.bfloat16, kind="ExternalInput")
    w_in = nc.dram_tensor("wcat", [H, WALL], DT.bfloat16, kind="ExternalInput")
    o_out = nc.dram_tensor("out", [BC, T, H], DT.float32, kind="ExternalOutput")

    with ExitStack() as ctx:
        tc = ctx.enter_context(tile.TileContext(nc))
        const = ctx.enter_context(tc.tile_pool(name="const", bufs=1))
        fpool = ctx.enter_context(tc.tile_pool(name="fpool", bufs=3))
        xtpool = ctx.enter_context(tc.tile_pool(name="xtpool", bufs=2))
        trashp = ctx.enter_context(tc.tile_pool(name="trashp", bufs=2))
        relup = ctx.enter_context(tc.tile_pool(name="relup", bufs=4))
        v2pool = ctx.enter_context(tc.tile_pool(name="v2pool", bufs=6))
        xnpool = ctx.enter_context(tc.tile_pool(name="xnpool", bufs=2))
        tiny = ctx.enter_context(tc.tile_pool(name="tiny", bufs=3))
        outp = ctx.enter_context(tc.tile_pool(name="outp", bufs=2))
        gtmp = ctx.enter_context(tc.tile_pool(name="gtmp", bufs=2))
        pqk = ctx.enter_context(tc.tile_pool(name="pqk", bufs=2, space="PSUM"))
        pvm = ctx.enter_context(tc.tile_pool(name="pvm", bufs=1, space="PSUM"))

        # --- constants ---
        wsb = const.tile([128, NCH, WALL], DT.bfloat16)
        for ch in range(NCH):
            nc.sync.dma_start(out=wsb[:, ch, :], in_=w_in[ch * 128:(ch + 1) * 128, :])
        epst = const.tile([G, 1], DT.float32)
        nc.vector.memset(epst, EPS)

        for gi in range(groups):
            b0 = gi * G

            fg = fpool.tile([G, T, H], DT.float32, tag="fg")
            for t in range(T):
                nc.sync.dma_start(out=fg[:, t, 0:384], in_=f_in[b0:b0 + G, t, 0:384])
                nc.sync.dma_start(out=fg[:, t, 384:768], in_=f_in[b0:b0 + G, t, 384:768])

            # transposed bf16 features straight from DRAM via the DMA xbar:
            # xt[:, ch, :] = F_bf[b0:b0+128, :, ch*128:+128].T  ->  [128h, 384(b,t)]
            xt = xtpool.tile([128, NCH, T * G], DT.bfloat16, tag="xt")
            for ch in range(NCH):
                nc.sync.dma_start_transpose(
                    out=xt[:, ch, :],
                    in_=fb_in[b0:b0 + G, :, ch * 128:(ch + 1) * 128]
                    .rearrange("b t h -> (b t) h"))

            xn = xnpool.tile([G, T, H], DT.float32, tag="xn")
            sumsqs = tiny.tile([G, T], DT.float32, tag="sumsqs")
            r_cat = tiny.tile([G, T], DT.float32, tag="r_cat")

            qps, qns, kps, kns, v2s = [], [], [], [], []

            for t in range(T):
                # lhsT for this token: strided 128-col view of the xbar output
                xts = xt[:, :, :].rearrange("p c (b t) -> p c t b", t=T)

                # [Q1|K1] and [V21|mu] projections
                qk = pqk.tile([128, WQK], DT.float32, tag="qk")
                vm = pvm.tile([128, WVM], DT.float32, tag="vm")
                for ch in range(NCH):
                    lhsT = xts[:, ch, t, :]
                    for (n0, n1) in QK_SLICES:
                        nc.tensor.matmul(qk[:, n0:n1], lhsT, wsb[:, ch, n0:n1],
                                         start=(ch == 0), stop=(ch == NCH - 1))
                    for (n0, n1) in VM_SLICES:
                        nc.tensor.matmul(vm[:, n0:n1], lhsT,
                                         wsb[:, ch, WQK + n0:WQK + n1],
                                         start=(ch == 0), stop=(ch == NCH - 1))

                # sum of squares (ACT, output is trash) and LN scalars
                tsq = trashp.tile([G, H], DT.bfloat16, tag="tsq")
                nc.scalar.activation(out=tsq, in_=fg[:, t, :], func=ACT.Square,
                                     accum_out=sumsqs[:, t:t + 1])
                mu = tiny.tile([G, 1], DT.float32, tag="mu")
                nc.vector.tensor_copy(out=mu, in_=vm[:, H:H + 1])
                mm = tiny.tile([G, 1], DT.float32, tag="mm")
                nc.vector.tensor_mul(mm, mu, mu)
                var = tiny.tile([G, 1], DT.float32, tag="var")
                nc.vector.scalar_tensor_tensor(
                    out=var, in0=sumsqs[:, t:t + 1], scalar=1.0 / H, in1=mm,
                    op0=ALU.mult, op1=ALU.subtract)
                # r = rsqrt(var+eps) = exp(-0.5*ln(var+eps)); Ln+Exp share one
                # ACT table set with the softmax Exp (no table switches)
                lv = tiny.tile([G, 1], DT.float32, tag="lv")
                nc.scalar.activation(out=lv, in_=var, func=ACT.Ln,
                                     bias=epst, scale=1.0)
                nc.scalar.activation(out=r_cat[:, t:t + 1], in_=lv,
                                     func=ACT.Exp, scale=-0.5)
                nmur = tiny.tile([G, 1], DT.float32, tag="nmur")
                nc.vector.tensor_scalar(
                    out=nmur, in0=mu, scalar1=r_cat[:, t:t + 1], scalar2=-1.0,
                    op0=ALU.mult, op1=ALU.mult)
                # xn = F*r - mu*r  (fp32, residual path)
                nc.scalar.activation(
                    out=xn[:, t, :], in_=fg[:, t, :], func=ACT.Identity,
                    bias=nmur, scale=r_cat[:, t:t + 1])

                # relus straight out of PSUM (bf16 outputs for the S products)
                qp = relup.tile([G, H], DT.bfloat16, tag="qp")
                qn = relup.tile([G, H], DT.bfloat16, tag="qn")
                kp = relup.tile([G, H], DT.bfloat16, tag="kp")
                kn = relup.tile([G, H], DT.bfloat16, tag="kn")
                nc.vector.tensor_scalar_max(qp, qk[:, 0:H], 0.0)
                nc.vector.tensor_scalar(out=qn, in0=qk[:, 0:H], scalar1=-1.0,
                                        scalar2=0.0, op0=ALU.mult, op1=ALU.max)
                nc.vector.tensor_scalar_max(kp, qk[:, H:2 * H], 0.0)
                nc.vector.tensor_scalar(out=kn, in0=qk[:, H:2 * H], scalar1=-1.0,
                                        scalar2=0.0, op0=ALU.mult, op1=ALU.max)
                # V21 to SBUF (fp32)
                v2 = v2pool.tile([G, H], DT.float32, tag="v2")
                nc.scalar.copy(out=v2, in_=vm[:, 0:H])

                qps.append(qp); qns.append(qn); kps.append(kp); kns.append(kn)
                v2s.append(v2)

            # S products: s1p/s1n[b, 3i+j] = <qp_i, kp_j> / <qn_i, kn_j>
            s1p = tiny.tile([G, 9], DT.float32, tag="s1p")
            s1n = tiny.tile([G, 9], DT.float32, tag="s1n")
            for i in range(T):
                for j in range(T):
                    o = 3 * i + j
                    # elementwise products on gpsimd, row-sums via ACT accum
                    pp = trashp.tile([G, H], DT.bfloat16, tag="prodp")
                    nc.gpsimd.tensor_mul(pp, qps[i], kps[j])
                    nc.scalar.activation(out=pp, in_=pp, func=ACT.Copy,
                                         bias=0.0, accum_out=s1p[:, o:o + 1])
                    pn = trashp.tile([G, H], DT.bfloat16, tag="prodn")
                    nc.gpsimd.tensor_mul(pn, qns[i], kns[j])
                    nc.scalar.activation(out=pn, in_=pn, func=ACT.Copy,
                                         bias=0.0, accum_out=s1n[:, o:o + 1])

            # scale by r_i*r_j, exp, row-sum, normalize, subtract, fold r_j
            rr9 = tiny.tile([G, 9], DT.float32, tag="rr9")
            for i in range(T):
                nc.vector.tensor_scalar_mul(rr9[:, 3 * i:3 * i + 3], r_cat,
                                            r_cat[:, i:i + 1])
            sep = tiny.tile([G, 9], DT.float32, tag="sep")
            sen = tiny.tile([G, 9], DT.float32, tag="sen")
            nc.vector.tensor_mul(sep, s1p, rr9)
            nc.vector.tensor_mul(sen, s1n, rr9)
            ep = tiny.tile([G, 9], DT.float32, tag="ep")
            en = tiny.tile([G, 9], DT.float32, tag="en")
            nc.scalar.activation(out=ep, in_=sep, func=ACT.Exp)
            nc.scalar.activation(out=en, in_=sen, func=ACT.Exp)
            sump = tiny.tile([G, 3], DT.float32, tag="sump")
            sumnn = tiny.tile([G, 3], DT.float32, tag="sumnn")
            ep3 = ep.rearrange("p (i j) -> p i j", j=3)
            en3 = en.rearrange("p (i j) -> p i j", j=3)
            nc.vector.tensor_reduce(out=sump, in_=ep3, axis=mybir.AxisListType.X,
                                    op=ALU.add)
            nc.vector.tensor_reduce(out=sumnn, in_=en3, axis=mybir.AxisListType.X,
                                    op=ALU.add, negate=True)
            rcp = tiny.tile([G, 3], DT.float32, tag="rcp")
            rcn = tiny.tile([G, 3], DT.float32, tag="rcn")
            nc.vector.reciprocal(rcp, sump)
            nc.vector.reciprocal(rcn, sumnn)   # = -1/sumN
            d2 = tiny.tile([G, 9], DT.float32, tag="d2")
            for i in range(T):
                t1 = tiny.tile([G, 3], DT.float32, tag="t1")
                nc.vector.tensor_scalar_mul(t1, ep[:, 3 * i:3 * i + 3],
                                            rcp[:, i:i + 1])
                dpi = tiny.tile([G, 3], DT.float32, tag="dpi")
                nc.vector.scalar_tensor_tensor(
                    out=dpi, in0=en[:, 3 * i:3 * i + 3], scalar=rcn[:, i:i + 1],
                    in1=t1, op0=ALU.mult, op1=ALU.add)
                # fold r_j (for V2) -> final coefficients
                nc.vector.tensor_mul(d2[:, 3 * i:3 * i + 3], dpi, r_cat)

            # out_i = xn_i + sum_j d2[i,j] * v2_j
            outg = outp.tile([G, T, H], DT.float32, tag="outg")
            for i in range(T):
                a1 = gtmp.tile([G, H], DT.float32, tag="a1")
                nc.vector.scalar_tensor_tensor(
                    out=a1, in0=v2s[0], scalar=d2[:, 3 * i:3 * i + 1],
                    in1=xn[:, i, :], op0=ALU.mult, op1=ALU.add)
                a2 = gtmp.tile([G, H], DT.float32, tag="a2")
                nc.vector.scalar_tensor_tensor(
                    out=a2, in0=v2s[1], scalar=d2[:, 3 * i + 1:3 * i + 2],
                    in1=a1, op0=ALU.mult, op1=ALU.add)
                nc.vector.scalar_tensor_tensor(
                    out=outg[:, i, :], in0=v2s[2], scalar=d2[:, 3 * i + 2:3 * i + 3],
                    in1=a2, op0=ALU.mult, op1=ALU.add)

            for t in range(T):
                nc.sync.dma_start(out=o_out[b0:b0 + G, t, 0:384], in_=outg[:, t, 0:384])
                nc.sync.dma_start(out=o_out[b0:b0 + G, t, 384:768], in_=outg[:, t, 384:768])

    nc.compile()
    return nc


def prepare_weights(Wq, Wk, Wv, Wo, ln_g, ln_b, bq, bk, bv, bo):
    """Fold layernorm scale/centering, the H^-0.5 attention scale, and Wo
    into a single [H, 2306] bf16 weight matrix (last cols: mean, pad)."""
    g = np.asarray(ln_g, np.float64)
    b = np.asarray(ln_b, np.float64)
    Wq = np.asarray(Wq, np.float64); Wk = np.asarray(Wk, np.float64)
    Wv = np.asarray(Wv, np.float64); Wo = np.asarray(Wo, np.float64)
    s4 = float(H) ** -0.25

    # Q/K biases (incl. b_ln routed through the projections) must vanish for
    # the relu/rstd folding to be exact. They are zeros for this problem.
    bq_eff = Wq @ b + np.asarray(bq, np.float64)
    bk_eff = Wk @ b + np.asarray(bk, np.float64)
    assert np.abs(bq_eff).max() < 1e-6, "nonzero effective Q bias unsupported"
    assert np.abs(bk_eff).max() < 1e-6, "nonzero effective K bias unsupported"

    Aq = (Wq * g[None, :]).T * s4          # [h_in, h_out]
    Ak = (Wk * g[None, :]).T * s4
    W2 = Wo @ Wv                           # [o, h]
    Av = (W2 * g[None, :]).T               # [h_in, o]
    cat = np.concatenate([Aq, Ak, Av], axis=1)          # [H, 3H]
    cat = cat - cat.mean(axis=0, keepdims=True)         # fold LN centering
    mu_col = np.full((H, 1), 1.0 / H)
    pad = np.zeros((H, 1))
    cat = np.concatenate([cat, mu_col, pad], axis=1)    # [H, 2306]
    c_vec = b + np.asarray(bo, np.float64)              # residual const
    return cat.astype(ml_dtypes.bfloat16), c_vec.astype(np.float32)


_NC_CACHE = {}
LAST_RESULTS = None


def install_ntff_hook(so_path="/opt/axon/libaxon_pjrt.so"):
    """Register the axon NTFF profiling hook (this image's antenv lacks
    axon_hooks; replicate trn_boot's ctypes shim so trace=True works)."""
    import types, ctypes, contextlib
    import antenv

    if hasattr(antenv, "axon_hooks"):
        return
    lib = ctypes.CDLL(so_path)
    if not hasattr(lib, "axon_start_nrt_profile"):
        return
    lib.axon_start_nrt_profile.argtypes = [ctypes.POINTER(ctypes.c_int64),
                                           ctypes.c_size_t]
    lib.axon_start_nrt_profile.restype = ctypes.c_int64
    lib.axon_stop_nrt_profile.argtypes = [ctypes.c_char_p]
    lib.axon_stop_nrt_profile.restype = ctypes.c_int64

    @contextlib.contextmanager
    def _hook(output_dir, device_ids):
        import jax
        jax.devices()
        if device_ids:
            ids = (ctypes.c_int64 * len(device_ids))(*device_ids)
            rc = lib.axon_start_nrt_profile(ids, len(device_ids))
        else:
            rc = lib.axon_start_nrt_profile(None, 0)
        if rc != 0:
            raise RuntimeError(f"axon_start_nrt_profile rc={rc}")
        try:
            yield
        finally:
            n = lib.axon_stop_nrt_profile(str(output_dir).encode())
            print(f"profile: {n} file(s) written to {output_dir}")

    mod = types.ModuleType("antenv.axon_hooks")
    mod.get_axon_ntff_profile_hook = lambda: _hook
    mod.set_axon_ntff_profile_hook = lambda h: None
    sys.modules["antenv.axon_hooks"] = mod
    antenv.axon_hooks = mod


def kernel(**inputs):
    feats = np.ascontiguousarray(np.asarray(inputs["features"], np.float32))
    fbf = feats.astype(ml_dtypes.bfloat16)
    wcat, c_vec = prepare_weights(
        inputs["Wq"], inputs["Wk"], inputs["Wv"], inputs["Wo"],
        inputs["ln_g"], inputs["ln_b"],
        inputs["bq"], inputs["bk"], inputs["bv"], inputs["bo"])
    assert np.abs(c_vec).max() < 1e-6, "nonzero ln_b/bo unsupported"

    import os
    groups = int(os.environ.get("KERNEL_GROUPS", str(BC // G)))
    trace = os.environ.get("KERNEL_TRACE", "0") == "1"
    if groups not in _NC_CACHE:
        _NC_CACHE[groups] = build_nc(groups)
    nc = _NC_CACHE[groups]

    in_maps = [
        {"features": feats[i * BC:(i + 1) * BC],
         "fbf": fbf[i * BC:(i + 1) * BC],
         "wcat": wcat}
        for i in range(NCORES)
    ]
    res = run_bass_kernel_spmd(nc, in_maps, core_ids=list(range(NCORES)),
                               trace=trace)
    global LAST_RESULTS
    LAST_RESULTS = res
    out = np.concatenate([r["out"] for r in res.results], axis=0)
    return out


# revision 19
# speedup vs baseline: 1.1559x; 1.1559x over previous
"""PolaLinearAttention TRN2 kernel.

Math (reference):
    x  = LayerNorm(f)*g + b_ln                  [B,T,H], T=3, H=768
    Q  = x@Wq.T+bq ; K = x@Wk.T+bk ; V = x@Wv.T+bv
    Sp = softmax(relu(Q)relu(K).T * H^-.5) ; Sn = softmax(relu(-Q)relu(-K).T * H^-.5)
    out = x + (Sp - Sn)@V@Wo.T + bo

Device strategy (pure batch data-parallel over 8 cores), host folds:
    - g (ln scale) and H^-0.25 into projection weights
    - LN mean-centering into column-centered weights (rank-1 update)
    - Wo into Wv (V2 = V@Wo.T); V-path biases cancel through softmax row
      sums (sum_j Sp = sum_j Sn = 1)
    - per-token LN rstd r applied via relu positive-homogeneity:
      S[i,j] *= r_i*r_j after the QK dot products; r_j folded into the
      attention coefficients of the V2 combination
    - a bf16 copy of features is shipped so the feature transpose runs on
      the DMA xbar straight out of DRAM

Per 128-batch group: DMA F (fp32) + 6 xbar-transposed bf16 chunks ->
PE matmuls into single-bank PSUM slices (keeps PE dense / HAM warm) ->
ACT evacuates QK to bf16 SBUF -> DVE relus at 4x -> paired elementwise
products (DVE+gpsimd) -> row sums (DVE tensor_reduce + ACT accum) ->
softmax small ops -> bf16 scalar_tensor_tensor V2 combination seeded
with the bf16 residual -> DMA out fp32.
"""

import sys

sys.path.insert(0, "/opt/trn_rl_repo")

import numpy as np
import ml_dtypes
from contextlib import ExitStack

import concourse.bass as bass
import concourse.bacc as bacc
import concourse.tile as tile
from concourse import mybir
from concourse.bass_utils import run_bass_kernel_spmd

DT = mybir.dt
ALU = mybir.AluOpType
ACT = mybir.ActivationFunctionType

H = 768
T = 3
B = 65536
NCORES = 8
BC = B // NCORES  # 8192 batches per core
G = 128           # batches per group
EPS = 1e-5
NCH = H // 128    # 6 k-chunks
WALL = 3 * H      # 2304 weight columns: Q | K | V2
# single-PSUM-bank matmul slices
SLICES = [(0, 512), (512, 1024), (1024, 1536), (1536, 2048), (2048, 2304)]
NPROD_GPS = 5     # product pairs computed on gpsimd (rest DVE)
NRED_DVE = 4      # pair reductions on DVE tensor_reduce (rest ACT accum)


def build_nc(groups: int) -> bass.Bass:
    nc = bacc.Bacc(None)
    f_in = nc.dram_tensor("features", [BC, T, H], DT.float32, kind="ExternalInput")
    fb_in = nc.dram_tensor("fbf", [BC, T, H], DT.bfloat16, kind="ExternalInput")
    w_in = nc.dram_tensor("wcat", [H, WALL], DT.bfloat16, kind="ExternalInput")
    o_out = nc.dram_tensor("out", [BC, T, H], DT.float32, kind="ExternalOutput")

    with ExitStack() as ctx:
        tc = ctx.enter_context(tile.TileContext(nc))
        const = ctx.enter_context(tc.tile_pool(name="const", bufs=1))
        fpool = ctx.enter_context(tc.tile_pool(name="fpool", bufs=3))
        xtpool = ctx.enter_context(tc.tile_pool(name="xtpool", bufs=2))
        qkpool = ctx.enter_context(tc.tile_pool(name="qkpool", bufs=4))
        relup = ctx.enter_context(tc.tile_pool(name="relup", bufs=4))
        prodp = ctx.enter_context(tc.tile_pool(name="prodp", bufs=4))
        v2pool = ctx.enter_context(tc.tile_pool(name="v2pool", bufs=6))
        xnpool = ctx.enter_context(tc.tile_pool(name="xnpool", bufs=2))
        tiny = ctx.enter_context(tc.tile_pool(name="tiny", bufs=3))
        outp = ctx.enter_context(tc.tile_pool(name="outp", bufs=2))
        gtmp = ctx.enter_context(tc.tile_pool(name="gtmp", bufs=3))
        psum = ctx.enter_context(tc.tile_pool(name="psum", bufs=8, space="PSUM"))

        # --- constants ---
        wsb = const.tile([128, NCH, WALL], DT.bfloat16)
        for ch in range(NCH):
            nc.sync.dma_start(out=wsb[:, ch, :], in_=w_in[ch * 128:(ch + 1) * 128, :])
        epst = const.tile([G, 1], DT.float32)
        nc.vector.memset(epst, EPS)

        for gi in range(groups):
            b0 = gi * G

            fg = fpool.tile([G, T, H], DT.float32, tag="fg")
            for t in range(T):
                nc.sync.dma_start(out=fg[:, t, 0:384], in_=f_in[b0:b0 + G, t, 0:384])
                nc.sync.dma_start(out=fg[:, t, 384:768], in_=f_in[b0:b0 + G, t, 384:768])

            # transposed bf16 features from DRAM via the DMA xbar:
            # xt[:, ch, :] = F_bf[b0:b0+128, :, ch*128:+128].T  ->  [128h, 384(b,t)]
            xt = xtpool.tile([128, NCH, T * G], DT.bfloat16, tag="xt")
            for ch in range(NCH):
                nc.sync.dma_start_transpose(
                    out=xt[:, ch, :],
                    in_=fb_in[b0:b0 + G, :, ch * 128:(ch + 1) * 128]
                    .rearrange("b t h -> (b t) h"))
            xts = xt.rearrange("p c (b t) -> p c t b", t=T)

            xnb = xnpool.tile([G, T, H], DT.bfloat16, tag="xnb")
            r_cat = tiny.tile([G, T], DT.float32, tag="r_cat")

            qcats, kcats, v2s = [], [], []

            for t in range(T):
                # projections into 5 single-bank PSUM slice tiles
                ps = []
                for si, (n0, n1) in enumerate(SLICES):
                    ps.append(psum.tile([128, n1 - n0], DT.float32, tag="ps",
                                        name=f"ps{si}"))
                for ch in range(NCH):
                    lhsT = xts[:, ch, t, :]
                    for si, (n0, n1) in enumerate(SLICES):
                        nc.tensor.matmul(ps[si][:, :], lhsT, wsb[:, ch, n0:n1],
                                         start=(ch == 0), stop=(ch == NCH - 1))

                # evacuate QK (bf16) and V2 (bf16) via ACT
                qkbf = qkpool.tile([G, 2 * H], DT.bfloat16, tag="qkbf")
                nc.scalar.copy(out=qkbf[:, 0:512], in_=ps[0])
                nc.scalar.copy(out=qkbf[:, 512:1024], in_=ps[1])
                nc.scalar.copy(out=qkbf[:, 1024:1536], in_=ps[2])
                v2 = v2pool.tile([G, H], DT.bfloat16, tag="v2")
                nc.scalar.copy(out=v2[:, 0:512], in_=ps[3])
                nc.scalar.copy(out=v2[:, 512:768], in_=ps[4])

                # LN stats via bn_stats/bn_aggr (fp32)
                stats = tiny.tile([G, 2, 6], DT.float32, tag="stats")
                fgv = fg[:, t, :].rearrange("p (s d) -> p s d", s=2)
                nc.vector.bn_stats(out=stats[:, 0, :], in_=fgv[:, 0, :])
                nc.vector.bn_stats(out=stats[:, 1, :], in_=fgv[:, 1, :])
                mv = tiny.tile([G, 2], DT.float32, tag="mv")
                nc.vector.bn_aggr(out=mv, in_=stats)
                # r = rsqrt(var+eps) = exp(-0.5*ln(var+eps))
                lv = tiny.tile([G, 1], DT.float32, tag="lv")
                nc.scalar.activation(out=lv, in_=mv[:, 1:2], func=ACT.Ln,
                                     bias=epst, scale=1.0)
                nc.scalar.activation(out=r_cat[:, t:t + 1], in_=lv,
                                     func=ACT.Exp, scale=-0.5)
                nmur = tiny.tile([G, 1], DT.float32, tag="nmur")
                nc.vector.tensor_scalar(
                    out=nmur, in0=mv[:, 0:1], scalar1=r_cat[:, t:t + 1],
                    scalar2=-1.0, op0=ALU.mult, op1=ALU.mult)
                # xn = F*r - mu*r  (bf16 residual seed)
                nc.vector.tensor_scalar(
                    out=xnb[:, t, :], in0=fg[:, t, :], scalar1=r_cat[:, t:t + 1],
                    scalar2=nmur, op0=ALU.mult, op1=ALU.add)

                # relus at 4x from bf16 SBUF: qcat = [Qp|Qn], kcat = [Kp|Kn]
                qcat = relup.tile([G, 2, H], DT.bfloat16, tag="qcat")
                kcat = relup.tile([G, 2, H], DT.bfloat16, tag="kcat")
                nc.vector.tensor_scalar_max(qcat[:, 0, :], qkbf[:, 0:H], 0.0)
                nc.vector.tensor_scalar(out=qcat[:, 1, :], in0=qkbf[:, 0:H],
                                        scalar1=-1.0, scalar2=0.0,
                                        op0=ALU.mult, op1=ALU.max)
                nc.vector.tensor_scalar_max(kcat[:, 0, :], qkbf[:, H:2 * H], 0.0)
                nc.vector.tensor_scalar(out=kcat[:, 1, :], in0=qkbf[:, H:2 * H],
                                        scalar1=-1.0, scalar2=0.0,
                                        op0=ALU.mult, op1=ALU.max)

                qcats.append(qcat); kcats.append(kcat); v2s.append(v2)

            # S products (paired pos|neg) + row sums
            # s1f[b, 2*(3i+j)+0/1] = <qp_i,kp_j> / <qn_i,kn_j>
            s1f = tiny.tile([G, 18], DT.float32, tag="s1f")
            for i in range(T):
                for j in range(T):
                    o = 3 * i + j
                    prod = prodp.tile([G, 2, H], DT.bfloat16, tag="prod")
                    eng = nc.gpsimd if o < NPROD_GPS else nc.vector
                    eng.tensor_mul(prod, qcats[i], kcats[j])
                    if o < NRED_DVE:
                        nc.vector.tensor_reduce(
                            out=s1f[:, 2 * o:2 * o + 2], in_=prod,
                            axis=mybir.AxisListType.X, op=ALU.add)
                    else:
                        nc.scalar.activation(
                            out=prod[:, 0, :], in_=prod[:, 0, :], func=ACT.Copy,
                            bias=0.0, accum_out=s1f[:, 2 * o:2 * o + 1])
                        nc.scalar.activation(
                            out=prod[:, 1, :], in_=prod[:, 1, :], func=ACT.Copy,
                            bias=0.0, accum_out=s1f[:, 2 * o + 1:2 * o + 2])

            # scale by r_i*r_j, exp, row-sum, normalize, subtract, fold r_j
            rr9 = tiny.tile([G, 9], DT.float32, tag="rr9")
            for i in range(T):
                nc.scalar.activation(out=rr9[:, 3 * i:3 * i + 3], in_=r_cat,
                                     func=ACT.Copy, scale=r_cat[:, i:i + 1])
            s1v = s1f.rearrange("p (o w) -> p w o", w=2)
            se = tiny.tile([G, 18], DT.float32, tag="se")
            nc.vector.tensor_mul(se[:, 0:9], s1v[:, 0, :], rr9)
            nc.vector.tensor_mul(se[:, 9:18], s1v[:, 1, :], rr9)
            e18 = tiny.tile([G, 18], DT.float32, tag="e18")
            nc.scalar.activation(out=e18, in_=se, func=ACT.Exp)
            sump = tiny.tile([G, 3], DT.float32, tag="sump")
            sumnn = tiny.tile([G, 3], DT.float32, tag="sumnn")
            nc.vector.tensor_reduce(
                out=sump, in_=e18[:, 0:9].rearrange("p (i j) -> p i j", j=3),
                axis=mybir.AxisListType.X, op=ALU.add)
            nc.vector.tensor_reduce(
                out=sumnn, in_=e18[:, 9:18].rearrange("p (i j) -> p i j", j=3),
                axis=mybir.AxisListType.X, op=ALU.add, negate=True)
            rcp = tiny.tile([G, 3], DT.float32, tag="rcp")
            rcn = tiny.tile([G, 3], DT.float32, tag="rcn")
            nc.vector.reciprocal(rcp, sump)
            nc.vector.reciprocal(rcn, sumnn)   # = -1/sumN
            d2 = tiny.tile([G, 9], DT.float32, tag="d2")
            for i in range(T):
                t1 = tiny.tile([G, 3], DT.float32, tag="t1")
                nc.scalar.activation(out=t1, in_=e18[:, 3 * i:3 * i + 3],
                                     func=ACT.Copy, scale=rcp[:, i:i + 1])
                dpi = tiny.tile([G, 3], DT.float32, tag="dpi")
                nc.vector.scalar_tensor_tensor(
                    out=dpi, in0=e18[:, 9 + 3 * i:12 + 3 * i],
                    scalar=rcn[:, i:i + 1], in1=t1, op0=ALU.mult, op1=ALU.add)
                # fold r_j (for V2) -> final coefficients
                nc.vector.tensor_mul(d2[:, 3 * i:3 * i + 3], dpi, r_cat)

            # out_i = xn_i + sum_j d2[i,j] * v2_j   (bf16 chain, fp32 final)
            outg = outp.tile([G, T, H], DT.float32, tag="outg")
            for i in range(T):
                a1 = gtmp.tile([G, H], DT.bfloat16, tag="a1")
                nc.vector.scalar_tensor_tensor(
                    out=a1, in0=v2s[0], scalar=d2[:, 3 * i:3 * i + 1],
                    in1=xnb[:, i, :], op0=ALU.mult, op1=ALU.add)
                a2 = gtmp.tile([G, H], DT.bfloat16, tag="a2")
                nc.vector.scalar_tensor_tensor(
                    out=a2, in0=v2s[1], scalar=d2[:, 3 * i + 1:3 * i + 2],
                    in1=a1, op0=ALU.mult, op1=ALU.add)
                nc.vector.scalar_tensor_tensor(
                    out=outg[:, i, :], in0=v2s[2], scalar=d2[:, 3 * i + 2:3 * i + 3],
                    in1=a2, op0=ALU.mult, op1=ALU.add)

            for t in range(T):
                nc.sync.dma_start(out=o_out[b0:b0 + G, t, 0:384], in_=outg[:, t, 0:384])
                nc.sync.dma_start(out=o_out[b0:b0 + G, t, 384:768], in_=outg[:, t, 384:768])

    nc.compile()
    return nc


def prepare_weights(Wq, Wk, Wv, Wo, ln_g, ln_b, bq, bk, bv, bo):
    """Fold layernorm scale/centering, the H^-0.5 attention scale, and Wo
    into a single [H, 2304] bf16 weight matrix."""
    g = np.asarray(ln_g, np.float64)
    b = np.asarray(ln_b, np.float64)
    Wq = np.asarray(Wq, np.float64); Wk = np.asarray(Wk, np.float64)
    Wv = np.asarray(Wv, np.float64); Wo = np.asarray(Wo, np.float64)
    s4 = float(H) ** -0.25

    # Q/K biases (incl. b_ln routed through the projections) must vanish for
    # the relu/rstd folding to be exact. They are zeros for this problem.
    bq_eff = Wq @ b + np.asarray(bq, np.float64)
    bk_eff = Wk @ b + np.asarray(bk, np.float64)
    assert np.abs(bq_eff).max() < 1e-6, "nonzero effective Q bias unsupported"
    assert np.abs(bk_eff).max() < 1e-6, "nonzero effective K bias unsupported"

    Aq = (Wq * g[None, :]).T * s4          # [h_in, h_out]
    Ak = (Wk * g[None, :]).T * s4
    W2 = Wo @ Wv                           # [o, h]
    Av = (W2 * g[None, :]).T               # [h_in, o]
    cat = np.concatenate([Aq, Ak, Av], axis=1)          # [H, 3H]
    cat = cat - cat.mean(axis=0, keepdims=True)         # fold LN centering
    c_vec = b + np.asarray(bo, np.float64)              # residual const
    return cat.astype(ml_dtypes.bfloat16), c_vec.astype(np.float32)


_NC_CACHE = {}
LAST_RESULTS = None


def install_ntff_hook(so_path="/opt/axon/libaxon_pjrt.so"):
    """Register the axon NTFF profiling hook (this image's antenv lacks
    axon_hooks; replicate trn_boot's ctypes shim so trace=True works)."""
    import types, ctypes, contextlib
    import antenv

    if hasattr(antenv, "axon_hooks"):
        return
    lib = ctypes.CDLL(so_path)
    if not hasattr(lib, "axon_start_nrt_profile"):
        return
    lib.axon_start_nrt_profile.argtypes = [ctypes.POINTER(ctypes.c_int64),
                                           ctypes.c_size_t]
    lib.axon_start_nrt_profile.restype = ctypes.c_int64
    lib.axon_stop_nrt_profile.argtypes = [ctypes.c_char_p]
    lib.axon_stop_nrt_profile.restype = ctypes.c_int64

    @contextlib.contextmanager
    def _hook(output_dir, device_ids):
        import jax
        jax.devices()
        if device_ids:
            ids = (ctypes.c_int64 * len(device_ids))(*device_ids)
            rc = lib.axon_start_nrt_profile(ids, len(device_ids))
        else:
            rc = lib.axon_start_nrt_profile(None, 0)
        if rc != 0:
            raise RuntimeError(f"axon_start_nrt_profile rc={rc}")
        try:
            yield
        finally:
            n = lib.axon_stop_nrt_profile(str(output_dir).encode())
            print(f"profile: {n} file(s) written to {output_dir}")

    mod = types.ModuleType("antenv.axon_hooks")
    mod.get_axon_ntff_profile_hook = lambda: _hook
    mod.set_axon_ntff_profile_hook = lambda h: None
    sys.modules["antenv.axon_hooks"] = mod
    antenv.axon_hooks = mod


def kernel(**inputs):
    feats = np.ascontiguousarray(np.asarray(inputs["features"], np.float32))
    fbf = feats.astype(ml_dtypes.bfloat16)
    wcat, c_vec = prepare_weights(
        inputs["Wq"], inputs["Wk"], inputs["Wv"], inputs["Wo"],
        inputs["ln_g"], inputs["ln_b"],
        inputs["bq"], inputs["bk"], inputs["bv"], inputs["bo"])
    assert np.abs(c_vec).max() < 1e-6, "nonzero ln_b/bo unsupported"

    import os
    groups = int(os.environ.get("KERNEL_GROUPS", str(BC // G)))
    trace = os.environ.get("KERNEL_TRACE", "0") == "1"
    if groups not in _NC_CACHE:
        _NC_CACHE[groups] = build_nc(groups)
    nc = _NC_CACHE[groups]

    in_maps = [
        {"features": feats[i * BC:(i + 1) * BC],
         "fbf": fbf[i * BC:(i + 1) * BC],
         "wcat": wcat}
        for i in range(NCORES)
    ]
    res = run_bass_kernel_spmd(nc, in_maps, core_ids=list(range(NCORES)),
                               trace=trace)
    global LAST_RESULTS
    LAST_RESULTS = res
    out = np.concatenate([r["out"] for r in res.results], axis=0)
    return out
